# revision 1
# baseline (speedup 1.0000x reference)
"""Grouped linear (MoE expert GEMM) on 8 NeuronCores, expert-parallel.

Problem: hidden_states [16384, 2048] f32, weight [8, 2048, 2048] f32,
tokens_per_expert [8] = 2048 each (balanced). Output [16384, 2048] f32 with
out[g*2048+t, o] = sum_i x[g*2048+t, i] * weight[g, o, i].

Sharding: expert-parallel — core g gets expert g's weight [2048, 2048] and its
2048 routed tokens; each core runs one 2048x2048x2048 GEMM. No collectives.

Per-core kernel: fp32 data fed to the PE as float32r (4-xbus fp32 streaming,
1 cycle/row at moving-dim >= 256, i.e. full 128x128 MAC rate). X^T is held
fully resident in SBUF (16 tiles of [128, 16, 128]); W^T streams through in
four [128, 16, 512] chunks; PSUM accumulates over the 16 k-chunks of 128.
"""

import numpy as np

G = 8
TPG = 2048  # tokens per expert (= per core)
IN = 2048
OUT = 2048
P = 128
KM = IN // P  # 16 contraction chunks of 128
TT = TPG // P  # 16 token tiles of 128
ON = 4  # number of output-column chunks
OW = OUT // ON  # 512

_nc_cache = {}


def _build_nc():
    import concourse.bacc as bacc
    import concourse.mybir as mybir
    import concourse.tile as tile

    if "nc" in _nc_cache:
        return _nc_cache["nc"]

    f32 = mybir.dt.float32
    f32r = mybir.dt.float32r

    nc = bacc.Bacc(None, target_bir_lowering=False)

    # xt[p, tt, km, tl] = x_core[tt*128+tl, km*128+p]   (X^T, k on partitions)
    xt = nc.dram_tensor("xt", [P, TT, KM, P], f32r, kind="ExternalInput")
    # wt[p, km, o] = w_core[o, km*128+p]                (W^T, k on partitions)
    wt = nc.dram_tensor("wt", [P, KM, OUT], f32r, kind="ExternalInput")
    # out[tt, p, o] = C[tt*128+p, o]
    out = nc.dram_tensor("out", [TT, P, OUT], f32, kind="ExternalOutput")

    with tile.TileContext(nc) as tc:
        with (
            tc.tile_pool(name="xpool", bufs=1) as xpool,
            tc.tile_pool(name="wpool", bufs=2) as wpool,
            tc.tile_pool(name="opool", bufs=7) as opool,
            tc.tile_pool(name="ppool", bufs=8, space="PSUM") as ppool,
        ):
            # Whole X^T resident: 16 x 8KB/partition = 128KB/partition.
            xtiles = []
            for tt in range(TT):
                x_sb = xpool.tile(
                    [P, KM, P], f32r, name=f"x_sb{tt}", tag=f"x{tt}"
                )
                nc.sync.dma_start(out=x_sb[:], in_=xt[:, tt])
                xtiles.append(x_sb)
                if tt == 0:
                    w_sb0 = wpool.tile(
                        [P, KM, OW], f32r, name="w_sb0", tag="w"
                    )
                    nc.sync.dma_start(out=w_sb0[:], in_=wt[:, :, 0:OW])

            w_tiles = [w_sb0]
            for oi in range(ON):
                w_sb = w_tiles[oi]
                for tt in range(TT):
                    psum = ppool.tile([P, OW], f32, name="psum", tag="psum")
                    for km in range(KM):
                        nc.tensor.matmul(
                            out=psum[:],
                            lhsT=xtiles[tt][:, km, :],
                            rhs=w_sb[:, km, :],
                            start=(km == 0),
                            stop=(km == KM - 1),
                        )
                    if tt == 8 and oi + 1 < ON:
                        # Prefetch the next W chunk mid-sweep: late enough
                        # that this sweep's early output DMAs outrank it,
                        # early enough to land before the next sweep starts.
                        w_next = wpool.tile(
                            [P, KM, OW], f32r, name=f"w_sb{oi + 1}", tag="w"
                        )
                        nc.sync.dma_start(
                            out=w_next[:],
                            in_=wt[:, :, (oi + 1) * OW : (oi + 2) * OW],
                        )
                        w_tiles.append(w_next)
                    o_sb = opool.tile([P, OW], f32, name="o_sb", tag="o_sb")
                    nc.vector.tensor_copy(out=o_sb[:], in_=psum[:])
                    nc.sync.dma_start(
                        out=out[tt, :, oi * OW : (oi + 1) * OW], in_=o_sb[:]
                    )

    nc.compile()
    _nc_cache["nc"] = nc
    return nc


def _shard_inputs(hidden_states, weight):
    """Host-side reshuffle into the DRAM layouts the kernel expects."""
    x = np.ascontiguousarray(np.asarray(hidden_states, dtype=np.float32))
    w = np.ascontiguousarray(np.asarray(weight, dtype=np.float32))
    in_maps = []
    for g in range(G):
        xg = x[g * TPG : (g + 1) * TPG]  # [2048, 2048]
        # [tt, tl, km, p] -> [p, tt, km, tl]
        xt = np.ascontiguousarray(
            xg.reshape(TT, P, KM, P).transpose(3, 0, 2, 1)
        )
        wg = w[g]  # [out, in]
        # [o, km, p] -> [p, km, o]
        wt = np.ascontiguousarray(
            wg.reshape(OUT, KM, P).transpose(2, 1, 0)
        )
        in_maps.append({"xt": xt, "wt": wt})
    return in_maps


def _run(hidden_states, weight, trace=False, tmpdir=None):
    from concourse.bass_utils import run_bass_kernel_spmd

    nc = _build_nc()
    in_maps = _shard_inputs(hidden_states, weight)
    res = run_bass_kernel_spmd(
        nc, in_maps, core_ids=list(range(G)), trace=trace, tmpdir=tmpdir
    )
    outs = [
        np.asarray(res.results[g]["out"]).reshape(TPG, OUT) for g in range(G)
    ]
    full = np.concatenate(outs, axis=0)
    return full, res


def kernel(hidden_states, weight, tokens_per_expert=None, **_ignored):
    out, _ = _run(hidden_states, weight, trace=False)
    return out



# revision 2
# speedup vs baseline: 1.0798x; 1.0798x over previous
"""Grouped linear (MoE expert GEMM) on 8 NeuronCores, expert-parallel.

Problem: hidden_states [16384, 2048] f32, weight [8, 2048, 2048] f32,
tokens_per_expert [8] = 2048 each (balanced). Output [16384, 2048] f32 with
out[g*2048+t, o] = sum_i x[g*2048+t, i] * weight[g, o, i].

Sharding: expert-parallel — core g gets expert g's weight [2048, 2048] and its
2048 routed tokens; each core runs one 2048x2048x2048 GEMM. No collectives.

Per-core kernel: inputs are rounded to bf16 on the host (rel err ~2.4e-3 on
the output, vs the 2e-2 gate). bf16 streams the PE at 1 row/cycle with fast
weight load (FWL), so the steady state is one 512-col matmul every ~216 ns.
X^T and W^T are both fully resident in SBUF (64 KB/partition each). DMA is
split across the two HWDGE queues — W slices on the sync queue, X tiles and
outputs on the scalar queue — ordered so the first matmul's dependencies
(x tile 0, then W k-slices for output chunk 0 in consumption order) land
first, instead of behind the whole 16 MB input load.
"""

import numpy as np
import ml_dtypes

G = 8
TPG = 2048  # tokens per expert (= per core)
IN = 2048
OUT = 2048
P = 128
KM = IN // P  # 16 contraction chunks of 128
TT = TPG // P  # 16 token tiles of 128
ON = 4  # number of output-column chunks
OW = OUT // ON  # 512

_nc_cache = {}


def _build_nc():
    import concourse.bacc as bacc
    import concourse.mybir as mybir
    import concourse.tile as tile

    if "nc" in _nc_cache:
        return _nc_cache["nc"]

    f32 = mybir.dt.float32
    bf16 = mybir.dt.bfloat16

    nc = bacc.Bacc(None, target_bir_lowering=False)

    # xt[p, tt, km, tl] = x_core[tt*128+tl, km*128+p]   (X^T, k on partitions)
    xt = nc.dram_tensor("xt", [P, TT, KM, P], bf16, kind="ExternalInput")
    # wt[p, km, oi, o] = w_core[oi*512+o, km*128+p]     (W^T, k on partitions)
    wt = nc.dram_tensor("wt", [P, KM, ON, OW], bf16, kind="ExternalInput")
    # out[tt, p, o] = C[tt*128+p, o]
    out = nc.dram_tensor("out", [TT, P, OUT], f32, kind="ExternalOutput")

    with tile.TileContext(nc) as tc:
        with (
            tc.tile_pool(name="xpool", bufs=1) as xpool,
            tc.tile_pool(name="wpool", bufs=1) as wpool,
            tc.tile_pool(name="opool", bufs=8) as opool,
            tc.tile_pool(name="ppool", bufs=8, space="PSUM") as ppool,
        ):
            xtiles = [
                xpool.tile([P, KM, P], bf16, name=f"x_sb{tt}", tag=f"x{tt}")
                for tt in range(TT)
            ]
            wtiles = [
                [
                    wpool.tile([P, OW], bf16, name=f"w_sb{km}_{oi}", tag=f"w{km}_{oi}")
                    for oi in range(ON)
                ]
                for km in range(KM)
            ]

            # Critical path first: x tile 0 on the scalar HWDGE queue, the 16
            # W k-slices of output chunk 0 (consumed in km order by the first
            # accumulation group) on the sync HWDGE queue. Bulk follows.
            nc.scalar.dma_start(out=xtiles[0][:], in_=xt[:, 0])
            for km in range(KM):
                nc.sync.dma_start(out=wtiles[km][0][:], in_=wt[:, km, 0])
            for tt in range(1, TT):
                nc.scalar.dma_start(out=xtiles[tt][:], in_=xt[:, tt])
            for oi in range(1, ON):
                for km in range(KM):
                    nc.sync.dma_start(out=wtiles[km][oi][:], in_=wt[:, km, oi])

            for oi in range(ON):
                for tt in range(TT):
                    psum = ppool.tile([P, OW], f32, name="psum", tag="psum")
                    for km in range(KM):
                        nc.tensor.matmul(
                            out=psum[:],
                            lhsT=xtiles[tt][:, km, :],
                            rhs=wtiles[km][oi][:],
                            start=(km == 0),
                            stop=(km == KM - 1),
                        )
                    o_sb = opool.tile([P, OW], f32, name="o_sb", tag="o_sb")
                    nc.vector.tensor_copy(out=o_sb[:], in_=psum[:])
                    nc.scalar.dma_start(
                        out=out[tt, :, oi * OW : (oi + 1) * OW], in_=o_sb[:]
                    )

    nc.compile()
    _nc_cache["nc"] = nc
    return nc


def _shard_inputs(hidden_states, weight):
    """Host-side reshuffle + bf16 rounding into the kernel's DRAM layouts."""
    bf16 = ml_dtypes.bfloat16
    x = np.asarray(hidden_states, dtype=np.float32).astype(bf16)
    w = np.asarray(weight, dtype=np.float32).astype(bf16)
    in_maps = []
    for g in range(G):
        xg = x[g * TPG : (g + 1) * TPG]  # [2048, 2048]
        # [tt, tl, km, p] -> [p, tt, km, tl]
        xtg = np.ascontiguousarray(xg.reshape(TT, P, KM, P).transpose(3, 0, 2, 1))
        wg = w[g]  # [out, in]
        # [oi, o, km, p] -> [p, km, oi, o]
        wtg = np.ascontiguousarray(wg.reshape(ON, OW, KM, P).transpose(3, 2, 0, 1))
        in_maps.append({"xt": xtg, "wt": wtg})
    return in_maps


def _run(hidden_states, weight, trace=False, tmpdir=None):
    from concourse.bass_utils import run_bass_kernel_spmd

    nc = _build_nc()
    in_maps = _shard_inputs(hidden_states, weight)
    res = run_bass_kernel_spmd(
        nc, in_maps, core_ids=list(range(G)), trace=trace, tmpdir=tmpdir
    )
    outs = [
        np.asarray(res.results[g]["out"]).reshape(TPG, OUT) for g in range(G)
    ]
    full = np.concatenate(outs, axis=0)
    return full, res


def kernel(hidden_states, weight, tokens_per_expert=None, **_ignored):
    out, _ = _run(hidden_states, weight, trace=False)
    return out
